# revision 8
# baseline (speedup 1.0000x reference)
"""Multi-head attention (B=4, S=1024, D=1024, H=16) on 8 trn2 NeuronCores.

Sharding: core c = b*2 + g handles batch b and head-group g (8 heads, 512 of
the 1024 hidden dims). Data-parallel over B, tensor-parallel over heads.

Per-core device pipeline (everything in transposed [feature, seq] layout):
  - qT/kT = W[g-slice] @ X^T via PE (d-chunk-major, 8-bank PSUM residency),
    with +bias via K=1 matmuls and the 1/sqrt(hd) scale folded into qT.
  - per head: augmented tiles qT' = [qT/8; ones], kT' = [kT; maskbias] so a
    single K=65 matmul yields scores^T + mask with no extra pass.
  - exp on ScalarE; ctx^T and the softmax denominator Z in one accumulated
    matmul using [v | ones] as the stationary operand (M=65, row 64 = Z).
  - 1/Z broadcast across partitions via a K=1 ones matmul on PE; normalize
    attn^T and ctx^T on VectorE; attn^T streamed to DRAM.
  - output projection ctx^T-chunks @ Wo^T-slice, + bo via K=1 matmul.
Host: transpose attn^T per (b,h) and sum the two head-group partials of out.

All matmuls run as float32r (1 cycle/row on PE, ~1.5e-4 rel err vs fp32).
"""

import numpy as np

B, S, D, H = 4, 1024, 1024, 16
HD = D // H          # 64 head dim
G = 2                # head groups -> 8 cores = B * G
HG = H // G          # 8 heads per core
R = HG * HD          # 512 feature rows per core
NC = 128             # partitions
DC = D // NC         # 8 d-chunks
KT = S // NC         # 8 key tiles
QCH = 512            # query chunk (matmul free dim)
NQC = S // QCH       # 2 query chunks
RC = R // NC         # 4 head-pair chunks
NEG = -1e10
PREC = "bf16"   # "bf16" or "f32r" for the matmul datapath

_CACHE = {}


def _build():
    import concourse.mybir as mybir
    from concourse import bacc
    from concourse.tile import TileContext

    f32 = mybir.dt.float32
    f32r = mybir.dt.float32r
    mmd = mybir.dt.bfloat16 if PREC == "bf16" else f32r
    attn_dt = mybir.dt.bfloat16 if PREC == "bf16" else f32
    Exp = mybir.ActivationFunctionType.Exp
    Copy = mybir.ActivationFunctionType.Copy
    mult = mybir.AluOpType.mult

    nc = bacc.Bacc(None, target_bir_lowering=False)

    xqT = nc.dram_tensor("xqT", [D, S], mmd, kind="ExternalInput")
    xkT = nc.dram_tensor("xkT", [D, S], mmd, kind="ExternalInput")
    xvT = nc.dram_tensor("xvT", [D, S], mmd, kind="ExternalInput")
    wqT = nc.dram_tensor("wqT", [D, R], mmd, kind="ExternalInput")
    wkT = nc.dram_tensor("wkT", [D, R], mmd, kind="ExternalInput")
    wvT = nc.dram_tensor("wvT", [D, R], mmd, kind="ExternalInput")
    woT = nc.dram_tensor("woT", [R, D], mmd, kind="ExternalInput")
    maskb = nc.dram_tensor("maskb", [1, S], mmd, kind="ExternalInput")
    onesd = nc.dram_tensor("onesd", [1, S], mmd, kind="ExternalInput")
    bqd = nc.dram_tensor("bqd", [1, R], mmd, kind="ExternalInput")
    bkd = nc.dram_tensor("bkd", [1, R], mmd, kind="ExternalInput")
    bvd = nc.dram_tensor("bvd", [1, R], mmd, kind="ExternalInput")
    bod = nc.dram_tensor("bod", [1, D], mmd, kind="ExternalInput")
    attnT = nc.dram_tensor("attnT", [HG, S, S], attn_dt, kind="ExternalOutput")
    onesr = nc.dram_tensor("onesr", [1, NC], f32r, kind="ExternalInput")
    outp = nc.dram_tensor("outp", [S, D], f32, kind="ExternalOutput")

    from contextlib import ExitStack

    with TileContext(nc) as tc, ExitStack() as top:
        persist = top.enter_context(tc.tile_pool(name="persist", bufs=1))
        work = top.enter_context(tc.tile_pool(name="work", bufs=2))

        # ---- constants / biases in SBUF ----
        ones_sb = persist.tile([1, QCH], mmd, tag="ones")
        onesr_sb = persist.tile([1, NC], f32r, tag="onesr")
        nc.sync.dma_start(onesr_sb[:], onesr[:])
        nc.sync.dma_start(ones_sb[:], onesd[0:1, 0:QCH])
        bq_sb = persist.tile([1, R], mmd, tag="bq")
        nc.sync.dma_start(bq_sb[:], bqd[:])
        bk_sb = persist.tile([1, R], mmd, tag="bk")
        nc.sync.dma_start(bk_sb[:], bkd[:])
        bv_sb = persist.tile([1, R], mmd, tag="bv")
        nc.sync.dma_start(bv_sb[:], bvd[:])
        bo_sb = persist.tile([1, D], mmd, tag="bo")
        nc.sync.dma_start(bo_sb[:], bod[:])

        # ---- persistent activation tiles ----
        qT = [persist.tile([HD + 1, S], mmd, tag=f"qT{h}", name=f"qT{h}") for h in range(HG)]
        kT = [persist.tile([HD + 1, S], mmd, tag=f"kT{h}", name=f"kT{h}") for h in range(HG)]
        v_aug = [persist.tile([NC, HG, HD + 1], mmd, tag=f"vau{t}", name=f"vau{t}") for t in range(KT)]
        ctxT = [persist.tile([NC, S], mmd, tag=f"ctxT{p}", name=f"ctxT{p}") for p in range(RC)]

        # augmented rows: ones for qT', mask bias for kT'
        for h in range(HG):
            nc.sync.dma_start(qT[h][HD : HD + 1, :], onesd[:])
            nc.sync.dma_start(kT[h][HD : HD + 1, :], maskb[:])
        ones3d = onesd[0:1, :].rearrange("a (p t o) -> (a p) t o", p=NC, o=1)
        for t in range(KT):
            nc.sync.dma_start(v_aug[t][:, :, HD : HD + 1], ones3d)

        # ---- projections (d-chunk-major, 8 PSUM accumulators) ----
        with tc.tile_pool(name="pp", bufs=8, space="PSUM") as pp, tc.tile_pool(
            name="stream", bufs=3
        ) as stream:
            for which, xd, wd, b_sb in (
                ("q", xqT, wqT, bq_sb),
                ("k", xkT, wkT, bk_sb),
            ):
                ps = [pp.tile([NC, QCH], f32, tag="pp", name="pp") for _ in range(RC * NQC)]
                for dc in range(DC):
                    xt = stream.tile([NC, S], mmd, tag="x")
                    nc.sync.dma_start(xt[:], xd[dc * NC : (dc + 1) * NC, :])
                    wt = stream.tile([NC, R], mmd, tag="w")
                    nc.sync.dma_start(wt[:], wd[dc * NC : (dc + 1) * NC, :])
                    for rc in range(RC):
                        for sc in range(NQC):
                            nc.tensor.matmul(
                                ps[rc * NQC + sc][:],
                                wt[:, rc * NC : (rc + 1) * NC],
                                xt[:, sc * QCH : (sc + 1) * QCH],
                                start=(dc == 0),
                                stop=False,
                            )
                for rc in range(RC):
                    for sc in range(NQC):
                        nc.tensor.matmul(
                            ps[rc * NQC + sc][:],
                            b_sb[0:1, rc * NC : (rc + 1) * NC],
                            ones_sb[0:1, 0:QCH],
                            start=False,
                            stop=True,
                        )
                # split pair rows into per-head tiles (scale q by 1/8)
                for rc in range(RC):
                    for sc in range(NQC):
                        p = ps[rc * NQC + sc]
                        for j in range(2):
                            h = 2 * rc + j
                            dst = (qT if which == "q" else kT)[h][
                                0:HD, sc * QCH : (sc + 1) * QCH
                            ]
                            src = p[j * HD : (j + 1) * HD, :]
                            if which == "q":
                                nc.vector.tensor_scalar_mul(dst, src, 1.0 / np.sqrt(HD))
                            else:
                                nc.vector.tensor_copy(dst, src)

                del ps

            # v projection: natural [s, r] layout
            psv = [pp.tile([NC, R], f32, tag="pp", name="ppv") for _ in range(KT)]
            for dc in range(DC):
                xt = stream.tile([NC, S], mmd, tag="x")
                nc.sync.dma_start(xt[:], xvT[dc * NC : (dc + 1) * NC, :])
                wt = stream.tile([NC, R], mmd, tag="w")
                nc.sync.dma_start(wt[:], wvT[dc * NC : (dc + 1) * NC, :])
                for st in range(KT):
                    nc.tensor.matmul(
                        psv[st][:],
                        xt[:, st * NC : (st + 1) * NC],
                        wt[:],
                        start=(dc == 0),
                        stop=False,
                    )
            for st in range(KT):
                nc.tensor.matmul(
                    psv[st][:],
                    ones_sb[0:1, 0:NC],
                    bv_sb[0:1, :],
                    start=False,
                    stop=True,
                )
                nc.vector.tensor_copy(
                    v_aug[st][:, :, 0:HD],
                    psv[st][:].rearrange("p (h e) -> p h e", e=HD),
                )

        # ---- attention ----
        with (
            tc.tile_pool(name="psT", bufs=5, space="PSUM") as psT,
            tc.tile_pool(name="psCU", bufs=2, space="PSUM") as psCU,
            tc.tile_pool(name="psBC", bufs=1, space="PSUM") as psBC,
        ):
            for h in range(HG):
                for qc in range(NQC):
                    expT = work.tile([NC, KT, QCH], mmd, tag="expT")
                    pcu = psCU.tile([HD + 1, QCH], f32, tag="cu")
                    for kt in range(KT):
                        pst = psT.tile([NC, QCH], f32, tag="sT")
                        nc.tensor.matmul(
                            pst[:],
                            kT[h][:, kt * NC : (kt + 1) * NC],
                            qT[h][:, qc * QCH : (qc + 1) * QCH],
                            start=True,
                            stop=True,
                        )
                        nc.scalar.activation(expT[:, kt, :], pst[:], Exp)
                        nc.tensor.matmul(
                            pcu[:],
                            v_aug[kt][:, h, :],
                            expT[:, kt, :],
                            start=(kt == 0),
                            stop=(kt == KT - 1),
                        )
                    # 1/Z, broadcast across partitions via K=1 ones matmul
                    rz = work.tile([1, QCH], f32r, tag="rz")
                    with nc.allow_low_precision(reason="1/Z in f32r for PE broadcast"):
                        nc.vector.reciprocal(rz[:], pcu[HD : HD + 1, :])
                    pbc = psBC.tile([NC, QCH], f32, tag="bc")
                    nc.tensor.matmul(
                        pbc[:], onesr_sb[0:1, 0:NC], rz[:], start=True, stop=True
                    )
                    sbc = work.tile([NC, QCH], f32, tag="sbc")
                    nc.scalar.activation(sbc[:], pbc[:], Copy)
                    # normalized ctx^T into the pair tile
                    nc.vector.tensor_tensor(
                        ctxT[h // 2][
                            (h % 2) * HD : (h % 2 + 1) * HD,
                            qc * QCH : (qc + 1) * QCH,
                        ],
                        pcu[0:HD, :],
                        sbc[0:HD, :],
                        mult,
                    )
                    # normalized attn^T -> DRAM (in-place over expT, reread as f32)
                    for kt in range(KT):
                        nc.vector.tensor_tensor(
                            expT[:, kt, :], expT[:, kt, :], sbc[:], mult
                        )
                    nc.sync.dma_start(
                        attnT[h]
                        .rearrange("(t p) q -> p t q", p=NC)[
                            :, :, qc * QCH : (qc + 1) * QCH
                        ],
                        expT[:].bitcast(attn_dt),
                    )

        # ---- output projection ----
        with tc.tile_pool(name="psO", bufs=2, space="PSUM") as psO, tc.tile_pool(
            name="wostream", bufs=1
        ) as wos, tc.tile_pool(name="olate", bufs=2) as olate:
            wo_sb = [wos.tile([NC, D], mmd, tag=f"wo{p}", name=f"wo{p}") for p in range(RC)]
            for p in range(RC):
                nc.sync.dma_start(wo_sb[p][:], woT[p * NC : (p + 1) * NC, :])
            for qt in range(KT):
                ost = olate.tile([NC, D], f32, tag="ost")
                for oc in range(NQC):
                    po = psO.tile([NC, QCH], f32, tag="po")
                    for pc in range(RC):
                        nc.tensor.matmul(
                            po[:],
                            ctxT[pc][:, qt * NC : (qt + 1) * NC],
                            wo_sb[pc][:, oc * QCH : (oc + 1) * QCH],
                            start=(pc == 0),
                            stop=False,
                        )
                    nc.tensor.matmul(
                        po[:],
                        ones_sb[0:1, 0:NC],
                        bo_sb[0:1, oc * QCH : (oc + 1) * QCH],
                        start=False,
                        stop=True,
                    )
                    nc.vector.tensor_copy(ost[:, oc * QCH : (oc + 1) * QCH], po[:])
                nc.sync.dma_start(outp[qt * NC : (qt + 1) * NC, :], ost[:])

    nc.finalize()
    return nc


def _get_nc():
    if "nc" not in _CACHE:
        _CACHE["nc"] = _build()
    return _CACHE["nc"]


def kernel(query, key, value, mask, Wq, bq, Wk, bk, Wv, bv, Wo, bo, _trace=False):
    from concourse.bass_utils import run_bass_kernel_spmd
    import ml_dtypes

    f = np.float32
    md = ml_dtypes.bfloat16 if PREC == "bf16" else np.float32

    def cast(a):
        return np.ascontiguousarray(np.asarray(a).astype(md))
    query = np.asarray(query, f)
    key = np.asarray(key, f)
    value = np.asarray(value, f)
    mask = np.asarray(mask)
    Wq, bq = np.asarray(Wq, f), np.asarray(bq, f)
    Wk, bk = np.asarray(Wk, f), np.asarray(bk, f)
    Wv, bv = np.asarray(Wv, f), np.asarray(bv, f)
    Wo, bo = np.asarray(Wo, f), np.asarray(bo, f)

    nc = _get_nc()

    in_maps = []
    for c in range(B * G):
        b, g = divmod(c, G)
        rs = slice(g * R, (g + 1) * R)
        mb = np.where(mask[b, 0, 0, :] == 0, f(NEG), f(0.0))[None, :]
        in_maps.append(
            {
                "xqT": cast(query[b].T),
                "xkT": cast(key[b].T),
                "xvT": cast(value[b].T),
                "wqT": cast(Wq[rs, :].T),
                "wkT": cast(Wk[rs, :].T),
                "wvT": cast(Wv[rs, :].T),
                "woT": cast(Wo[:, rs].T),
                "maskb": cast(mb),
                "onesd": np.ones((1, S), md),
                "onesr": np.ones((1, NC), f),
                "bqd": cast(bq[rs][None, :]),
                "bkd": cast(bk[rs][None, :]),
                "bvd": cast(bv[rs][None, :]),
                "bod": cast((bo if g == 0 else np.zeros_like(bo))[None, :]),
            }
        )

    res = run_bass_kernel_spmd(nc, in_maps, core_ids=list(range(B * G)), trace=_trace)
    _CACHE["last_results"] = res

    out = np.empty((B, S, D), f)
    attn = np.empty((B, H, S, S), f)
    for c in range(B * G):
        b, g = divmod(c, G)
        r = res.results[c]
        if g == 0:
            out[b] = r["outp"]
        else:
            out[b] += r["outp"]
        attn[b, g * HG : (g + 1) * HG] = r["attnT"].transpose(0, 2, 1).astype(f)
    return out, attn


# revision 11
# speedup vs baseline: 1.5955x; 1.5955x over previous
"""Multi-head attention (B=4, S=1024, D=1024, H=16) on 8 trn2 NeuronCores.

Sharding: core c = b*2 + g handles batch b and head-group g (8 heads = 512 of
the 1024 hidden dims). Data-parallel over B, tensor-parallel over heads.

Per-core device pipeline (transposed [feature, seq] layout throughout):
  - qT/kT = W[g-slice] @ X^T on PE (d-chunk-major, 8 PSUM accumulators),
    biases via K=1 matmuls, the 1/sqrt(hd) scale folded into qT's copy-out.
    q/k stored as 4 head-pair tiles [128, S] (head 2i rows 0-63, 2i+1 64-127).
  - scores^T per (head, k-tile): K=64 matmuls; the two heads of a pair are
    issued adjacently with disjoint PE row-groups (base partition 0/64) so
    they run concurrently in the array.
  - exp on ScalarE over [128, 1024] PSUM tiles with the attention mask as a
    per-partition additive bias (-1e10 on masked keys -> exp = 0 exactly).
  - ctx^T and the softmax denominator Z in one accumulated matmul per head
    using [v | ones] as the stationary operand (M=65; row 64 = Z).
  - Z broadcast across partitions via a K=1 ones matmul on PE, then
    reciprocal_approx_fast on VectorE; ctx^T normalized on-chip.
  - UNNORMALIZED exp(scores^T) (bf16) and Z (fp32) stream to DRAM; the host
    divides + transposes while assembling the fp32 attn output.
  - output projection: ctx^T-pair-chunks @ Wo^T-slice, + bo via K=1 matmul.
Host: attn[b,h] = (expT / Z).T per head; out[b] = sum of the 2 group partials.
"""

import numpy as np

B, S, D, H = 4, 1024, 1024, 16
HD = D // H          # 64 head dim
G = 2                # head groups -> 8 cores = B * G
HG = H // G          # 8 heads per core
R = HG * HD          # 512 feature rows per core
NC = 128             # partitions
DC = D // NC         # 8 d-chunks
KT = S // NC         # 8 key tiles
QCH = 512            # matmul moving free dim (PSUM bank)
NQC = S // QCH       # 2 query chunks
RC = R // NC         # 4 head-pair chunks
NEG = -1e10
PREC = "bf16"        # matmul datapath dtype: "bf16" or "f32r"

_CACHE = {}


def _build():
    import concourse.mybir as mybir
    from concourse import bacc
    from concourse.tile import TileContext
    from contextlib import ExitStack

    f32 = mybir.dt.float32
    f32r = mybir.dt.float32r
    mmd = mybir.dt.bfloat16 if PREC == "bf16" else f32r
    Exp = mybir.ActivationFunctionType.Exp
    Copy = mybir.ActivationFunctionType.Copy
    mult = mybir.AluOpType.mult

    nc = bacc.Bacc(None, target_bir_lowering=False)

    xqT = nc.dram_tensor("xqT", [D, S], mmd, kind="ExternalInput")
    xkT = nc.dram_tensor("xkT", [D, S], mmd, kind="ExternalInput")
    xvT = nc.dram_tensor("xvT", [D, S], mmd, kind="ExternalInput")
    wqT = nc.dram_tensor("wqT", [D, R], mmd, kind="ExternalInput")
    wkT = nc.dram_tensor("wkT", [D, R], mmd, kind="ExternalInput")
    wvT = nc.dram_tensor("wvT", [D, R], mmd, kind="ExternalInput")
    woT = nc.dram_tensor("woT", [R, D], mmd, kind="ExternalInput")
    maskb = nc.dram_tensor("maskb", [1, S], f32, kind="ExternalInput")
    onesd = nc.dram_tensor("onesd", [1, S], mmd, kind="ExternalInput")
    onesr = nc.dram_tensor("onesr", [1, NC], f32r, kind="ExternalInput")
    bqd = nc.dram_tensor("bqd", [1, R], mmd, kind="ExternalInput")
    bkd = nc.dram_tensor("bkd", [1, R], mmd, kind="ExternalInput")
    bvd = nc.dram_tensor("bvd", [1, R], mmd, kind="ExternalInput")
    bod = nc.dram_tensor("bod", [1, D], mmd, kind="ExternalInput")
    # unnormalized exp(scores^T) per head [k, q], and Z per head [q]
    eout = nc.dram_tensor("eout", [HG, S, S], mmd, kind="ExternalOutput")
    zout = nc.dram_tensor("zout", [HG, S], f32r, kind="ExternalOutput")
    outp = nc.dram_tensor("outp", [S, D], f32, kind="ExternalOutput")

    with TileContext(nc) as tc, ExitStack() as top:
        persist = top.enter_context(tc.tile_pool(name="persist", bufs=1))
        work = top.enter_context(tc.tile_pool(name="work", bufs=2))

        # ---- constants / biases ----
        ones_sb = persist.tile([1, QCH], mmd, tag="ones")
        nc.sync.dma_start(ones_sb[:], onesd[0:1, 0:QCH])
        onesr_sb = persist.tile([1, NC], f32r, tag="onesr")
        nc.sync.dma_start(onesr_sb[:], onesr[:])
        maskb_sb = persist.tile([NC, KT], f32, tag="maskb")
        nc.sync.dma_start(
            maskb_sb[:], maskb[0:1, :].rearrange("a (t p) -> (a p) t", p=NC)
        )
        bq_sb = persist.tile([1, R], mmd, tag="bq")
        nc.sync.dma_start(bq_sb[:], bqd[:])
        bk_sb = persist.tile([1, R], mmd, tag="bk")
        nc.sync.dma_start(bk_sb[:], bkd[:])
        bv_sb = persist.tile([1, R], mmd, tag="bv")
        nc.sync.dma_start(bv_sb[:], bvd[:])
        bo_sb = persist.tile([1, D], mmd, tag="bo")
        nc.sync.dma_start(bo_sb[:], bod[:])

        # ---- persistent activations ----
        qTp = [persist.tile([NC, S], mmd, tag=f"qTp{p}", name=f"qTp{p}") for p in range(RC)]
        kTp = [persist.tile([NC, S], mmd, tag=f"kTp{p}", name=f"kTp{p}") for p in range(RC)]
        v_aug = [persist.tile([NC, HG, HD + 1], mmd, tag=f"vau{t}", name=f"vau{t}") for t in range(KT)]
        ctxT = [persist.tile([NC, S], mmd, tag=f"ctxT{p}", name=f"ctxT{p}") for p in range(RC)]

        ones3d = onesd[0:1, :].rearrange("a (p t o) -> (a p) t o", p=NC, o=1)
        for t in range(KT):
            nc.sync.dma_start(v_aug[t][:, :, HD : HD + 1], ones3d)

        # ---- projections (d-chunk-major, 8 PSUM accumulators) ----
        with tc.tile_pool(name="pp", bufs=8, space="PSUM") as pp, tc.tile_pool(
            name="stream", bufs=3
        ) as stream:
            for which, xd, wd, b_sb in (
                ("q", xqT, wqT, bq_sb),
                ("k", xkT, wkT, bk_sb),
            ):
                ps = [pp.tile([NC, QCH], f32, tag="pp", name="pp") for _ in range(RC * NQC)]
                for dc in range(DC):
                    xt = stream.tile([NC, S], mmd, tag="x")
                    nc.sync.dma_start(xt[:], xd[dc * NC : (dc + 1) * NC, :])
                    wt = stream.tile([NC, R], mmd, tag="w")
                    nc.sync.dma_start(wt[:], wd[dc * NC : (dc + 1) * NC, :])
                    for rc in range(RC):
                        for sc in range(NQC):
                            nc.tensor.matmul(
                                ps[rc * NQC + sc][:],
                                wt[:, rc * NC : (rc + 1) * NC],
                                xt[:, sc * QCH : (sc + 1) * QCH],
                                start=(dc == 0),
                                stop=False,
                            )
                for rc in range(RC):
                    for sc in range(NQC):
                        nc.tensor.matmul(
                            ps[rc * NQC + sc][:],
                            b_sb[0:1, rc * NC : (rc + 1) * NC],
                            ones_sb[0:1, 0:QCH],
                            start=False,
                            stop=True,
                        )
                for rc in range(RC):
                    for sc in range(NQC):
                        dst = (qTp if which == "q" else kTp)[rc][
                            :, sc * QCH : (sc + 1) * QCH
                        ]
                        if which == "q":
                            nc.vector.tensor_scalar_mul(
                                dst, ps[rc * NQC + sc][:], 1.0 / float(np.sqrt(HD))
                            )
                        else:
                            nc.vector.tensor_copy(dst, ps[rc * NQC + sc][:])
                del ps

            # v projection: natural [s, r] layout
            psv = [pp.tile([NC, R], f32, tag="pp", name="ppv") for _ in range(KT)]
            for dc in range(DC):
                xt = stream.tile([NC, S], mmd, tag="x")
                nc.sync.dma_start(xt[:], xvT[dc * NC : (dc + 1) * NC, :])
                wt = stream.tile([NC, R], mmd, tag="w")
                nc.sync.dma_start(wt[:], wvT[dc * NC : (dc + 1) * NC, :])
                for st in range(KT):
                    nc.tensor.matmul(
                        psv[st][:],
                        xt[:, st * NC : (st + 1) * NC],
                        wt[:],
                        start=(dc == 0),
                        stop=False,
                    )
            for st in range(KT):
                nc.tensor.matmul(
                    psv[st][:],
                    ones_sb[0:1, 0:NC],
                    bv_sb[0:1, :],
                    start=False,
                    stop=True,
                )
                nc.vector.tensor_copy(
                    v_aug[st][:, :, 0:HD],
                    psv[st][:].rearrange("p (h e) -> p h e", e=HD),
                )

        # ---- attention: per head-pair, full q width ----
        with (
            tc.tile_pool(name="psT", bufs=2, space="PSUM") as psT,
            tc.tile_pool(name="psCU", bufs=1, space="PSUM") as psCU,
            tc.tile_pool(name="psBC", bufs=2, space="PSUM") as psBC,
        ):
            for hp in range(RC):
                expT = [
                    work.tile([NC, KT, S], mmd, tag=f"expT{j}", name=f"expT{j}")
                    for j in range(2)
                ]
                # scores^T + exp: both heads of the pair issued adjacently so
                # their K=64 matmuls pack into disjoint PE row groups.
                for kt in range(KT):
                    pst = [
                        psT.tile([NC, NQC, QCH], f32, tag="sT", name="sT")
                        for _ in range(2)
                    ]
                    for j in range(2):
                        rows = slice(j * HD, (j + 1) * HD)
                        for sc in range(NQC):
                            nc.tensor.matmul(
                                pst[j][:, sc, :],
                                kTp[hp][rows, kt * NC : (kt + 1) * NC],
                                qTp[hp][rows, sc * QCH : (sc + 1) * QCH],
                                start=True,
                                stop=True,
                            )
                    for j in range(2):
                        nc.scalar.activation(
                            expT[j][:, kt, :],
                            pst[j][:],
                            Exp,
                            bias=maskb_sb[:, kt : kt + 1],
                        )
                for j in range(2):
                    h = 2 * hp + j
                    # ctx^T (+Z in row 64) accumulated over k tiles
                    pcu = psCU.tile([HD + 1, NQC, QCH], f32, tag="cu", name="cu")
                    for sc in range(NQC):
                        for kt in range(KT):
                            nc.tensor.matmul(
                                pcu[:, sc, :],
                                v_aug[kt][:, h, :],
                                expT[j][:, kt, sc * QCH : (sc + 1) * QCH],
                                start=(kt == 0),
                                stop=(kt == KT - 1),
                            )
                    # Z to SBUF (ACT copy), then DRAM (host normalizes attn)
                    zrow = work.tile([1, S], f32r, tag="zrow")
                    nc.scalar.activation(
                        zrow[:].rearrange("p (a q) -> p a q", a=NQC),
                        pcu[HD : HD + 1, :],
                        Copy,
                    )
                    nc.sync.dma_start(zout[h : h + 1, :], zrow[:])
                    pbc = psBC.tile([NC, QCH], f32, tag="bc", name="bc")
                    sbc = work.tile([NC, S], f32, tag="sbc")
                    for sc in range(NQC):
                        nc.tensor.matmul(
                            pbc[:],
                            onesr_sb[0:1, 0:NC],
                            zrow[0:1, sc * QCH : (sc + 1) * QCH],
                            start=True,
                            stop=True,
                        )
                        nc.vector.reciprocal_approx_fast(
                            out=sbc[:, sc * QCH : (sc + 1) * QCH], in_=pbc[:]
                        )
                        pbc = psBC.tile([NC, QCH], f32, tag="bc", name="bc")
                    # normalized ctx^T into the pair tile
                    nc.vector.tensor_tensor(
                        ctxT[hp][j * HD : (j + 1) * HD, :],
                        pcu[0:HD, :].rearrange("p a q -> p (a q)"),
                        sbc[0:HD, :],
                        mult,
                    )
                    # unnormalized exp(scores^T) -> DRAM
                    nc.sync.dma_start(
                        eout[h].rearrange("(t p) q -> p t q", p=NC), expT[j][:]
                    )

        # ---- output projection ----
        with tc.tile_pool(name="psO", bufs=2, space="PSUM") as psO, tc.tile_pool(
            name="wostream", bufs=1
        ) as wos, tc.tile_pool(name="olate", bufs=2) as olate:
            wo_sb = [wos.tile([NC, D], mmd, tag=f"wo{p}", name=f"wo{p}") for p in range(RC)]
            for p in range(RC):
                nc.sync.dma_start(wo_sb[p][:], woT[p * NC : (p + 1) * NC, :])
            for qt in range(KT):
                ost = olate.tile([NC, D], f32, tag="ost")
                for oc in range(NQC):
                    po = psO.tile([NC, QCH], f32, tag="po", name="po")
                    for pc in range(RC):
                        nc.tensor.matmul(
                            po[:],
                            ctxT[pc][:, qt * NC : (qt + 1) * NC],
                            wo_sb[pc][:, oc * QCH : (oc + 1) * QCH],
                            start=(pc == 0),
                            stop=False,
                        )
                    nc.tensor.matmul(
                        po[:],
                        ones_sb[0:1, 0:NC],
                        bo_sb[0:1, oc * QCH : (oc + 1) * QCH],
                        start=False,
                        stop=True,
                    )
                    nc.vector.tensor_copy(ost[:, oc * QCH : (oc + 1) * QCH], po[:])
                nc.sync.dma_start(outp[qt * NC : (qt + 1) * NC, :], ost[:])

    nc.finalize()
    return nc


def _get_nc():
    if "nc" not in _CACHE:
        _CACHE["nc"] = _build()
    return _CACHE["nc"]


def kernel(query, key, value, mask, Wq, bq, Wk, bk, Wv, bv, Wo, bo, _trace=False):
    from concourse.bass_utils import run_bass_kernel_spmd
    import ml_dtypes

    f = np.float32
    md = ml_dtypes.bfloat16 if PREC == "bf16" else np.float32

    def cast(a):
        return np.ascontiguousarray(np.asarray(a).astype(md))

    query = np.asarray(query, f)
    key = np.asarray(key, f)
    value = np.asarray(value, f)
    mask = np.asarray(mask)
    Wq, bq = np.asarray(Wq, f), np.asarray(bq, f)
    Wk, bk = np.asarray(Wk, f), np.asarray(bk, f)
    Wv, bv = np.asarray(Wv, f), np.asarray(bv, f)
    Wo, bo = np.asarray(Wo, f), np.asarray(bo, f)

    nc = _get_nc()

    in_maps = []
    for c in range(B * G):
        b, g = divmod(c, G)
        rs = slice(g * R, (g + 1) * R)
        mb = np.where(mask[b, 0, 0, :] == 0, f(NEG), f(0.0))[None, :]
        in_maps.append(
            {
                "xqT": cast(query[b].T),
                "xkT": cast(key[b].T),
                "xvT": cast(value[b].T),
                "wqT": cast(Wq[rs, :].T),
                "wkT": cast(Wk[rs, :].T),
                "wvT": cast(Wv[rs, :].T),
                "woT": cast(Wo[:, rs].T),
                "maskb": np.ascontiguousarray(mb),
                "onesd": np.ones((1, S), md),
                "onesr": np.ones((1, NC), f),
                "bqd": cast(bq[rs][None, :]),
                "bkd": cast(bk[rs][None, :]),
                "bvd": cast(bv[rs][None, :]),
                "bod": cast((bo if g == 0 else np.zeros_like(bo))[None, :]),
            }
        )

    res = run_bass_kernel_spmd(nc, in_maps, core_ids=list(range(B * G)), trace=_trace)
    _CACHE["last_results"] = res

    out = np.empty((B, S, D), f)
    attn = np.empty((B, H, S, S), f)
    for c in range(B * G):
        b, g = divmod(c, G)
        r = res.results[c]
        if g == 0:
            out[b] = r["outp"]
        else:
            out[b] += r["outp"]
        e = r["eout"]  # [HG, S(k), S(q)] unnormalized exp, bf16
        z = r["zout"]  # [HG, S(q)] fp32
        for h in range(HG):
            # attn[q, k] = e[k, q] / Z[q]
            a = e[h].astype(f).T
            a /= z[h][:, None]
            attn[b, g * HG + h] = a
    return out, attn


# revision 12
# speedup vs baseline: 1.6971x; 1.0636x over previous
"""Multi-head attention (B=4, S=1024, D=1024, H=16) on 8 trn2 NeuronCores.

Sharding: core c = b*2 + g handles batch b and head-group g (8 heads = 512 of
the 1024 hidden dims). Data-parallel over B, tensor-parallel over heads.

Per-core device pipeline (transposed [feature, seq] layout throughout):
  - qT/kT = W[g-slice] @ X^T on PE (d-chunk-major, 8 PSUM accumulators),
    biases via K=1 matmuls, the 1/sqrt(hd) scale folded into qT's copy-out.
    q/k stored as 4 head-pair tiles [128, S] (head 2i rows 0-63, 2i+1 64-127).
  - scores^T per (head, k-tile): K=64 matmuls; the two heads of a pair are
    issued adjacently with disjoint PE row-groups (base partition 0/64) so
    they run concurrently in the array.
  - exp on ScalarE over [128, 1024] PSUM tiles with the attention mask as a
    per-partition additive bias (-1e10 on masked keys -> exp = 0 exactly).
  - ctx^T and the softmax denominator Z in one accumulated matmul per head
    using [v | ones] as the stationary operand (M=65; row 64 = Z).
  - Z broadcast across partitions via a K=1 ones matmul on PE, then
    reciprocal_approx_fast on VectorE; ctx^T normalized on-chip.
  - UNNORMALIZED exp(scores^T) (bf16) and Z (fp32) stream to DRAM; the host
    divides + transposes while assembling the fp32 attn output.
  - output projection: ctx^T-pair-chunks @ Wo^T-slice, + bo via K=1 matmul.
Host: attn[b,h] = (expT / Z).T per head; out[b] = sum of the 2 group partials.
"""

import numpy as np

B, S, D, H = 4, 1024, 1024, 16
HD = D // H          # 64 head dim
G = 2                # head groups -> 8 cores = B * G
HG = H // G          # 8 heads per core
R = HG * HD          # 512 feature rows per core
NC = 128             # partitions
DC = D // NC         # 8 d-chunks
KT = S // NC         # 8 key tiles
QCH = 512            # matmul moving free dim (PSUM bank)
NQC = S // QCH       # 2 query chunks
RC = R // NC         # 4 head-pair chunks
NEG = -1e10
PREC = "bf16"        # matmul datapath dtype: "bf16" or "f32r"

_CACHE = {}


def _build():
    import concourse.mybir as mybir
    from concourse import bacc
    from concourse.tile import TileContext
    from contextlib import ExitStack

    f32 = mybir.dt.float32
    f32r = mybir.dt.float32r
    mmd = mybir.dt.bfloat16 if PREC == "bf16" else f32r
    Exp = mybir.ActivationFunctionType.Exp
    Copy = mybir.ActivationFunctionType.Copy
    mult = mybir.AluOpType.mult

    nc = bacc.Bacc(None, target_bir_lowering=False)

    xqT = nc.dram_tensor("xqT", [D, S], mmd, kind="ExternalInput")
    xkT = nc.dram_tensor("xkT", [D, S], mmd, kind="ExternalInput")
    xvT = nc.dram_tensor("xvT", [D, S], mmd, kind="ExternalInput")
    wqT = nc.dram_tensor("wqT", [D, R], mmd, kind="ExternalInput")
    wkT = nc.dram_tensor("wkT", [D, R], mmd, kind="ExternalInput")
    wvT = nc.dram_tensor("wvT", [D, R], mmd, kind="ExternalInput")
    woT = nc.dram_tensor("woT", [R, D], mmd, kind="ExternalInput")
    maskb = nc.dram_tensor("maskb", [1, S], f32, kind="ExternalInput")
    onesd = nc.dram_tensor("onesd", [1, S], mmd, kind="ExternalInput")
    bqd = nc.dram_tensor("bqd", [1, R], mmd, kind="ExternalInput")
    bkd = nc.dram_tensor("bkd", [1, R], mmd, kind="ExternalInput")
    bvd = nc.dram_tensor("bvd", [1, R], mmd, kind="ExternalInput")
    bod = nc.dram_tensor("bod", [1, D], mmd, kind="ExternalInput")
    # unnormalized exp(scores^T) per head [k, q], and Z per head [q]
    eout = nc.dram_tensor("eout", [HG, S, S], mmd, kind="ExternalOutput")
    zout = nc.dram_tensor("zout", [HG, S], f32, kind="ExternalOutput")
    outp = nc.dram_tensor("outp", [S, D], f32, kind="ExternalOutput")

    with TileContext(nc) as tc, ExitStack() as top:
        persist = top.enter_context(tc.tile_pool(name="persist", bufs=1))
        work = top.enter_context(tc.tile_pool(name="work", bufs=2))

        # ---- constants / biases ----
        ones_sb = persist.tile([1, QCH], mmd, tag="ones")
        nc.gpsimd.dma_start(ones_sb[:], onesd[0:1, 0:QCH])
        maskb_sb = persist.tile([NC, KT], f32, tag="maskb")
        nc.gpsimd.dma_start(
            maskb_sb[:], maskb[0:1, :].rearrange("a (t p) -> (a p) t", p=NC)
        )
        bq_sb = persist.tile([1, R], mmd, tag="bq")
        nc.gpsimd.dma_start(bq_sb[:], bqd[:])
        bk_sb = persist.tile([1, R], mmd, tag="bk")
        nc.gpsimd.dma_start(bk_sb[:], bkd[:])
        bv_sb = persist.tile([1, R], mmd, tag="bv")
        nc.gpsimd.dma_start(bv_sb[:], bvd[:])
        bo_sb = persist.tile([1, D], mmd, tag="bo")
        nc.gpsimd.dma_start(bo_sb[:], bod[:])

        # ---- persistent activations ----
        qTp = [persist.tile([NC, S], mmd, tag=f"qTp{p}", name=f"qTp{p}") for p in range(RC)]
        kTp = [persist.tile([NC, S], mmd, tag=f"kTp{p}", name=f"kTp{p}") for p in range(RC)]
        v_aug = [persist.tile([NC, HG, HD + 1], mmd, tag=f"vau{t}", name=f"vau{t}") for t in range(KT)]
        ctxT = [persist.tile([NC, S], mmd, tag=f"ctxT{p}", name=f"ctxT{p}") for p in range(RC)]

        ones3d = onesd[0:1, :].rearrange("a (p t o) -> (a p) t o", p=NC, o=1)
        for t in range(KT):
            nc.gpsimd.dma_start(v_aug[t][:, :, HD : HD + 1], ones3d)

        # ---- projections (rc-major, resident x/w, overlap-friendly) ----
        pp = top.enter_context(tc.tile_pool(name="pp", bufs=2, space="PSUM"))
        stream = top.enter_context(tc.tile_pool(name="stream", bufs=2))

        # v projection first (attention needs v_aug for every pair)
        xt = stream.tile([NC, DC, S], mmd, tag="x")
        nc.sync.dma_start(xt[:], xvT.rearrange("(c p) s -> p c s", p=NC))
        wt = stream.tile([NC, DC, R], mmd, tag="w")
        nc.sync.dma_start(wt[:], wvT.rearrange("(c p) s -> p c s", p=NC))
        for st in range(KT):
            ps = pp.tile([NC, QCH], f32, tag="pp", name="pp")
            for dc in range(DC):
                nc.tensor.matmul(
                    ps[:],
                    xt[:, dc, st * NC : (st + 1) * NC],
                    wt[:, dc, :],
                    start=(dc == 0),
                    stop=False,
                )
            nc.tensor.matmul(
                ps[:], ones_sb[0:1, 0:NC], bv_sb[0:1, :], start=False, stop=True
            )
            nc.vector.tensor_copy(
                v_aug[st][:, :, 0:HD], ps[:].rearrange("p (h e) -> p h e", e=HD)
            )

        for which, xd, wd, b_sb in (
            ("k", xkT, wkT, bk_sb),
            ("q", xqT, wqT, bq_sb),
        ):
            xt = stream.tile([NC, DC, S], mmd, tag="x")
            nc.sync.dma_start(xt[:], xd.rearrange("(c p) s -> p c s", p=NC))
            wt = stream.tile([NC, DC, R], mmd, tag="w")
            nc.sync.dma_start(wt[:], wd.rearrange("(c p) s -> p c s", p=NC))
            for rc in range(RC):
                for sc in range(NQC):
                    ps = pp.tile([NC, QCH], f32, tag="pp", name="pp")
                    for dc in range(DC):
                        nc.tensor.matmul(
                            ps[:],
                            wt[:, dc, rc * NC : (rc + 1) * NC],
                            xt[:, dc, sc * QCH : (sc + 1) * QCH],
                            start=(dc == 0),
                            stop=False,
                        )
                    nc.tensor.matmul(
                        ps[:],
                        b_sb[0:1, rc * NC : (rc + 1) * NC],
                        ones_sb[0:1, 0:QCH],
                        start=False,
                        stop=True,
                    )
                    dst = (qTp if which == "q" else kTp)[rc][
                        :, sc * QCH : (sc + 1) * QCH
                    ]
                    if which == "q":
                        nc.vector.tensor_scalar_mul(
                            dst, ps[:], 1.0 / float(np.sqrt(HD))
                        )
                    else:
                        nc.vector.tensor_copy(dst, ps[:])

        # ---- attention: per head-pair, full q width ----
        with (
            tc.tile_pool(name="psT", bufs=2, space="PSUM") as psT,
            tc.tile_pool(name="psCU", bufs=2, space="PSUM") as psCU,
        ):
            for hp in range(RC):
                expT = [
                    work.tile([NC, KT, S], mmd, tag=f"expT{j}", name=f"expT{j}")
                    for j in range(2)
                ]
                # scores^T + exp: both heads of the pair issued adjacently so
                # their K=64 matmuls pack into disjoint PE row groups.
                for kt in range(KT):
                    pst = [
                        psT.tile([NC, NQC, QCH], f32, tag="sT", name="sT")
                        for _ in range(2)
                    ]
                    for j in range(2):
                        rows = slice(j * HD, (j + 1) * HD)
                        for sc in range(NQC):
                            nc.tensor.matmul(
                                pst[j][:, sc, :],
                                kTp[hp][rows, kt * NC : (kt + 1) * NC],
                                qTp[hp][rows, sc * QCH : (sc + 1) * QCH],
                                start=True,
                                stop=True,
                            )
                    for j in range(2):
                        nc.scalar.activation(
                            expT[j][:, kt, :],
                            pst[j][:],
                            Exp,
                            bias=maskb_sb[:, kt : kt + 1],
                        )
                for j in range(2):
                    h = 2 * hp + j
                    # ctx^T (+Z in row 64) accumulated over k tiles
                    zrow = work.tile([1, S], f32, tag="zrow")
                    pcus = []
                    for sc in range(NQC):
                        pcu = psCU.tile([HD + 1, QCH], f32, tag="cu", name="cu")
                        pcus.append(pcu)
                        for kt in range(KT):
                            nc.tensor.matmul(
                                pcu[:],
                                v_aug[kt][:, h, :],
                                expT[j][:, kt, sc * QCH : (sc + 1) * QCH],
                                start=(kt == 0),
                                stop=(kt == KT - 1),
                            )
                        nc.scalar.activation(
                            zrow[0:1, sc * QCH : (sc + 1) * QCH],
                            pcu[HD : HD + 1, :],
                            Copy,
                        )
                    nc.gpsimd.dma_start(zout[h : h + 1, :], zrow[:])
                    # broadcast Z across partitions (GpSimd), 1/Z on VectorE
                    zb = work.tile([NC, S], f32, tag="zb")
                    nc.gpsimd.partition_broadcast(zb[:], zrow[:])
                    sbc = work.tile([NC, S], f32, tag="sbc")
                    nc.vector.reciprocal_approx_fast(out=sbc[:], in_=zb[:])
                    # normalized ctx^T into the pair tile
                    for sc in range(NQC):
                        nc.vector.tensor_tensor(
                            ctxT[hp][j * HD : (j + 1) * HD, sc * QCH : (sc + 1) * QCH],
                            pcus[sc][0:HD, :],
                            sbc[0:HD, sc * QCH : (sc + 1) * QCH],
                            mult,
                        )
                    # unnormalized exp(scores^T) -> DRAM
                    nc.sync.dma_start(
                        eout[h].rearrange("(t p) q -> p t q", p=NC), expT[j][:]
                    )

        # ---- output projection ----
        with tc.tile_pool(name="psO", bufs=2, space="PSUM") as psO, tc.tile_pool(
            name="wostream", bufs=1
        ) as wos, tc.tile_pool(name="olate", bufs=2) as olate:
            wo_sb = [wos.tile([NC, D], mmd, tag=f"wo{p}", name=f"wo{p}") for p in range(RC)]
            for p in range(RC):
                nc.sync.dma_start(wo_sb[p][:], woT[p * NC : (p + 1) * NC, :])
            for qt in range(KT):
                ost = olate.tile([NC, D], f32, tag="ost")
                for oc in range(NQC):
                    po = psO.tile([NC, QCH], f32, tag="po", name="po")
                    for pc in range(RC):
                        nc.tensor.matmul(
                            po[:],
                            ctxT[pc][:, qt * NC : (qt + 1) * NC],
                            wo_sb[pc][:, oc * QCH : (oc + 1) * QCH],
                            start=(pc == 0),
                            stop=False,
                        )
                    nc.tensor.matmul(
                        po[:],
                        ones_sb[0:1, 0:NC],
                        bo_sb[0:1, oc * QCH : (oc + 1) * QCH],
                        start=False,
                        stop=True,
                    )
                    nc.vector.tensor_copy(ost[:, oc * QCH : (oc + 1) * QCH], po[:])
                nc.sync.dma_start(outp[qt * NC : (qt + 1) * NC, :], ost[:])

    nc.finalize()
    return nc


def _get_nc():
    if "nc" not in _CACHE:
        _CACHE["nc"] = _build()
    return _CACHE["nc"]


def kernel(query, key, value, mask, Wq, bq, Wk, bk, Wv, bv, Wo, bo, _trace=False):
    from concourse.bass_utils import run_bass_kernel_spmd
    import ml_dtypes

    f = np.float32
    md = ml_dtypes.bfloat16 if PREC == "bf16" else np.float32

    def cast(a):
        return np.ascontiguousarray(np.asarray(a).astype(md))

    query = np.asarray(query, f)
    key = np.asarray(key, f)
    value = np.asarray(value, f)
    mask = np.asarray(mask)
    Wq, bq = np.asarray(Wq, f), np.asarray(bq, f)
    Wk, bk = np.asarray(Wk, f), np.asarray(bk, f)
    Wv, bv = np.asarray(Wv, f), np.asarray(bv, f)
    Wo, bo = np.asarray(Wo, f), np.asarray(bo, f)

    nc = _get_nc()

    in_maps = []
    for c in range(B * G):
        b, g = divmod(c, G)
        rs = slice(g * R, (g + 1) * R)
        mb = np.where(mask[b, 0, 0, :] == 0, f(NEG), f(0.0))[None, :]
        in_maps.append(
            {
                "xqT": cast(query[b].T),
                "xkT": cast(key[b].T),
                "xvT": cast(value[b].T),
                "wqT": cast(Wq[rs, :].T),
                "wkT": cast(Wk[rs, :].T),
                "wvT": cast(Wv[rs, :].T),
                "woT": cast(Wo[:, rs].T),
                "maskb": np.ascontiguousarray(mb),
                "onesd": np.ones((1, S), md),
                "bqd": cast(bq[rs][None, :]),
                "bkd": cast(bk[rs][None, :]),
                "bvd": cast(bv[rs][None, :]),
                "bod": cast((bo if g == 0 else np.zeros_like(bo))[None, :]),
            }
        )

    res = run_bass_kernel_spmd(nc, in_maps, core_ids=list(range(B * G)), trace=_trace)
    _CACHE["last_results"] = res

    out = np.empty((B, S, D), f)
    attn = np.empty((B, H, S, S), f)
    for c in range(B * G):
        b, g = divmod(c, G)
        r = res.results[c]
        if g == 0:
            out[b] = r["outp"]
        else:
            out[b] += r["outp"]
        e = r["eout"]  # [HG, S(k), S(q)] unnormalized exp, bf16
        z = r["zout"]  # [HG, S(q)] fp32
        for h in range(HG):
            # attn[q, k] = e[k, q] / Z[q]
            a = e[h].astype(f).T
            a /= z[h][:, None]
            attn[b, g * HG + h] = a
    return out, attn
